# revision 55
# baseline (speedup 1.0000x reference)
"""Self-contained TRN2 Bass kernel for nn_MultiHeadAttention_77833397338481.

kernel(**inputs) takes the FULL unsharded inputs (Q, K, V [2,1024,1024],
Wq/Wk/Wv/Wo [1024,1024], biases [1024]) and returns the FULL output
[2, 1024, 1024]. 8 NeuronCores = batch(2) x head-group(4).

Design: the softmax exp stream (16.8M exps/core) is split between the
Scalar engine (ACT spline exp) and the Vector engine (Schraudolph
bit-trick exp: one tensor_scalar -> int16 bitcast as bf16, bias-centered
so its mean log-error is zero and mixed exact/approx rows stay unbiased).
 - bf16 matmul operands, fp32 PSUM accumulation; fp8 q/k projection path.
 - scores^T tiles stream through a 5-bank PSUM ring (A=[128,1536],
   B=[128,1024]); one fused-scale exp per chunk on ACT or DVE.
 - ctx matmuls 4-way column-tiled (tile_position=(0,32j)); V padded to
   32-col head slots with a ones column producing softmax denominators
   in the same matmuls; denominators read back via a partition-strided
   AP (no DMA), reciprocal on DVE.
 - batched input DMAs (one or two per tensor, 3 DGE rings) so the first
   exp fires ~9us in; warm-up matmul burst opens the HAM clock gate.
 - projections + n0 output projection are paced "fill" work inside the
   attention phase; n1 output projection forms a short tail with
   [128,1024] output DMAs split across rings.
"""

import math
from collections import deque

import numpy as np
import ml_dtypes

import concourse.bacc as bacc
import concourse.mybir as mybir
import concourse.tile as tile

F32 = mybir.dt.float32
FP8 = mybir.dt.float8e4
F32R = mybir.dt.float32r
BF16 = mybir.dt.bfloat16
I16 = mybir.dt.int16
AF = mybir.ActivationFunctionType
ALU = mybir.AluOpType

D = 1024
S = 1024
B = 2
E = 16
NQ = 4          # quads per core
NJ = 4          # heads per quad
ND = 8          # 128-row d chunks
NKB = 8         # 128-key blocks
SCALE = 1.0 / 32.0
NSL = 2 * NQ * NKB * NJ   # 256 score slices of [128 keys, 512 queries]

# --- DVE Schraudolph exp constants (bf16-bitcast domain) ---
# i16 = round(x * A16 + B16); bitcast bf16 ~= exp(SCALE*x) * (1 + eps(frac)),
# C_MAGIC chosen so E[ln(1+f-c) - f ln2] = 0 over f~U[0,1): eps is zero-mean.
C_MAGIC = 5.0
A16 = SCALE * math.log2(math.e) * 128.0
B16 = 127 * 128.0 - C_MAGIC
# exp engine split: ACT owns the psA ring (even chunks, 1536 cols = 60%),
# DVE owns the psB ring (odd chunks, 1024 cols = 40%) — the two 2-deep
# PSUM rings then never wait on each other's engine.
def dve_chunk(c):
    return c % 2 == 1


def bf16_np(x):
    return np.ascontiguousarray(x, np.float32).astype(ml_dtypes.bfloat16)


def fp8_np(x):
    return np.ascontiguousarray(x, np.float32).astype(ml_dtypes.float8_e4m3)


def round_fp32r(x):
    u = np.ascontiguousarray(x, np.float32).view(np.uint32)
    r = ((u.astype(np.uint64) + 0x800) & 0xFFFFF000).astype(np.uint32)
    return r.view(np.float32)


def chunk_of(s):
    """Global slice s -> (chunk id, position, nominal size). A chunks (even)
    hold 3 slices, B chunks (odd) hold 2."""
    pair, w = divmod(s, 5)
    if w < 3:
        return 2 * pair, w, 3
    return 2 * pair + 1, w - 3, 2


def build_nc():
    nc = bacc.Bacc("TRN2", target_bir_lowering=False, debug=False, num_devices=8)

    # inputs are host-prepacked into the exact SBUF layouts (partition-major,
    # d-chunk-contiguous) so every load is one contiguous [128, N] DMA
    xq_d = nc.dram_tensor("xq", [128, 8192], FP8, kind="ExternalInput")
    xk_d = nc.dram_tensor("xk", [128, 8192], FP8, kind="ExternalInput")
    xv_d = nc.dram_tensor("xv", [128, 8192], BF16, kind="ExternalInput")
    wqt_d = nc.dram_tensor("wqt", [128, 4096], FP8, kind="ExternalInput")
    wkt_d = nc.dram_tensor("wkt", [128, 4096], FP8, kind="ExternalInput")
    wvt_d = nc.dram_tensor("wvt", [128, 2176], BF16, kind="ExternalInput")
    wot_d = nc.dram_tensor("wot", [128, 4096], BF16, kind="ExternalInput")
    bvrow_d = nc.dram_tensor("bvrow", [1, 272], F32, kind="ExternalInput")
    bqp_d = nc.dram_tensor("bqp", [128, NQ], F32, kind="ExternalInput")
    bkp_d = nc.dram_tensor("bkp", [128, NQ], F32, kind="ExternalInput")
    ind_d = nc.dram_tensor("ind", [4, 128], F32R, kind="ExternalInput")
    sel_d = nc.dram_tensor("sel", [128, 4], F32R, kind="ExternalInput")
    out_d = nc.dram_tensor("out_part", [S, D], BF16, kind="ExternalOutput")

    with tile.TileContext(nc) as tc:
        with (
            tc.tile_pool(name="persist", bufs=1) as pp,
            tc.tile_pool(name="attn", bufs=1) as ap_,
            tc.tile_pool(name="psum", space="PSUM", bufs=1) as ps,
        ):
            # --- warm-up data (no DMA dependency): PE burst from ~0.4us so the
            # HAM clock gate opens before the first projections ---
            wub = pp.tile([128, 512], BF16, name="wub")
            nc.vector.memset(wub, 1.0)
            ones1 = pp.tile([1, 128], F32, name="ones1")
            nc.vector.memset(ones1, 1.0)
            wu = ps.tile([128, 512], F32, name="wu", tag="proj", bufs=2)
            for _ in range(7):
                nc.tensor.matmul(wu, wub[:, 0:128], wub, start=True, stop=True)

            # --- batched input DMAs: one per tensor(-half), d-major layout,
            # split across the three DGE rings (gpsimd=SWDGE k/v path,
            # sync=SP-HWDGE q path, scalar=ACT-HWDGE weights) in
            # deadline-priority order ---
            wkt = pp.tile([128, 8 * 512], FP8, name="wkt")
            wqt = pp.tile([128, 8 * 512], FP8, name="wqt")
            wvt = pp.tile([128, 8 * 272], BF16, name="wvt")
            wot_sb = pp.tile([128, 4 * 1024], BF16, name="wot")
            xk = [pp.tile([128, 8 * 512], FP8, name=f"xk{h}") for h in range(2)]
            xq = [pp.tile([128, 8 * 512], FP8, name=f"xq{h}") for h in range(2)]
            xv = [pp.tile([128, 8 * 512], BF16, name=f"xv{h}") for h in range(2)]

            # HBM bandwidth (~358 GB/s) is the lead-in limiter, and each DGE
            # ring is FIFO: put ONLY the first-chunk critical set (wkt/xk0,
            # then wqt/xq0) at the heads of the two HWDGE rings so it gets the
            # full bandwidth; everything later queues behind ON THE SAME rings.
            # The slow-starting SWDGE ring gets only tiny late-need consts.
            bq_sb = pp.tile([128, NQ], F32, name="bq_sb")
            bk_sb = pp.tile([128, NQ], F32, name="bk_sb")
            bvrow_sb = pp.tile([1, 272], F32, name="bvrow_sb")
            ind_sb = pp.tile([4, 128], F32R, name="ind_sb")
            sel_sb = pp.tile([128, 4], F32R, name="sel_sb")
            # W tensors are t-major so the first quad's weights (128KB each)
            # arrive in ~1us; the critical first-chunk set is then just
            # wkt_t0+xk0 / wqt_t0+xq0
            # critical first-chunk set (1.28MB) alone on the two fast HWDGE
            # rings; everything else rides the slow-starting SWDGE ring or
            # queues behind on sync, in deadline order
            nc.scalar.dma_start(out=wkt[:, 0:1024], in_=wkt_d[:, 0:1024])
            nc.scalar.dma_start(out=wqt[:, 0:1024], in_=wqt_d[:, 0:1024])
            nc.sync.dma_start(out=bq_sb, in_=bqp_d[:])
            nc.sync.dma_start(out=bk_sb, in_=bkp_d[:])
            nc.sync.dma_start(out=xk[0], in_=xk_d[:, 0:4096])
            nc.sync.dma_start(out=xq[0], in_=xq_d[:, 0:4096])
            nc.scalar.dma_start(out=wkt[:, 1024:4096], in_=wkt_d[:, 1024:4096])
            nc.scalar.dma_start(out=wqt[:, 1024:4096], in_=wqt_d[:, 1024:4096])
            nc.gpsimd.dma_start(out=wvt, in_=wvt_d[:])
            nc.gpsimd.dma_start(out=ind_sb, in_=ind_d[:])
            nc.gpsimd.dma_start(out=sel_sb, in_=sel_d[:])
            nc.gpsimd.dma_start(out=bvrow_sb, in_=bvrow_d[:])
            nc.sync.dma_start(out=xk[1], in_=xk_d[:, 4096:8192])
            nc.sync.dma_start(out=xv[0], in_=xv_d[:, 0:4096])
            nc.sync.dma_start(out=xv[1], in_=xv_d[:, 4096:8192])
            nc.sync.dma_start(out=xq[1], in_=xq_d[:, 4096:8192])
            nc.gpsimd.dma_start(out=wot_sb, in_=wot_d[:])
            # pre-load the ACT exp table during the lead-in (after the DMA
            # issues -- the ~1.3us table load must not delay them)
            actw = ap_.tile([1, 128], BF16, name="actw", tag="actw", bufs=1)
            nc.scalar.activation(actw, ones1, AF.Exp, scale=SCALE)

            # slice helpers
            def xk_v(d, h):
                return xk[h][:, 512 * d : 512 * (d + 1)]

            def xq_v(d, h):
                return xq[h][:, 512 * d : 512 * (d + 1)]

            def xv_v(d, h, q):
                return xv[h][:, 512 * d + 128 * q : 512 * d + 128 * (q + 1)]

            def wk_v(d, t):
                return wkt[:, 1024 * t + 128 * d : 1024 * t + 128 * (d + 1)]

            def wq_v(d, t):
                return wqt[:, 1024 * t + 128 * d : 1024 * t + 128 * (d + 1)]

            def wv_v(d):
                return wvt[:, 272 * d : 272 * (d + 1)]

            def wot_v(c, dc):
                return wot_sb[:, 1024 * c + 512 * dc : 1024 * c + 512 * (dc + 1)]

            # --- persistent activations ---
            qt = [pp.tile([128, S], BF16, name=f"qt{t}") for t in range(NQ)]
            kt = [pp.tile([128, S], BF16, name=f"kt{t}") for t in range(NQ)]
            va = [pp.tile([128, 512], BF16, name=f"va{s}") for s in range(NKB)]
            for sb in range(NKB):
                # gpsimd memsets: the DVE must be free for the first evacs
                nc.gpsimd.memset(va[sb], 0.0)
            ctxp = [pp.tile([128, S], BF16, name=f"ctxp{c}") for c in range(4)]

            # --- psum ring for scores/exp (5 banks) ---
            psA = ps.tile([128, 1536], F32, name="psA", tag="psA", bufs=1)
            psB = ps.tile([128, 1024], F32, name="psB", tag="psB", bufs=1)

            biasB = pp.tile([128, 272], F32, name="biasB")

            # emitted-producer tracking: a consumer emitted before its
            # producer would silently read stale/garbage SBUF on hardware
            done = set()

            # Deferred-evacuation queue: a PSUM->SBUF evacuation emitted right
            # after its producing matmuls would sit at the head of the ACT/DVE
            # queue waiting on the PE and stall the exp stream behind it.
            # Emitters push their evac closure here; it is emitted one fill
            # slot (~2 chunks) later, when the producer has already executed.
            evac_q = deque()

            def pump_evacs():
                while evac_q:
                    evac_q.popleft()()

            # ============ projection emitters ============
            DR = mybir.MatmulPerfMode.DoubleRow

            def qk_pair(w_t, dp, t):
                """[K=128, Ko=2, M=128] DoubleRow weight AP for d-pair dp
                (t-major layout: the d-pair is 256 contiguous cols)."""
                v = w_t[:, 1024 * t + 256 * dp : 1024 * t + 256 * (dp + 1)]
                return v.rearrange("p (i c) -> p i c", i=2)

            def x_pair(x_h, dp):
                """[K=128, Ko=2, N=512] DoubleRow moving AP for d-pair dp."""
                v = x_h[:, 1024 * dp : 1024 * (dp + 1)]
                return v.rearrange("p (i c) -> p i c", i=2)

            def proj_qk(which, t, h, defer=True):
                """q/k projection for quad t, s-half h -> qt/kt[t][:, 512h:].
                fp8 DoubleRow: 4 matmuls of 2 d-chunks each."""
                w_t, x_h, bias, dst = (
                    (wqt, xq[h], bq_sb, qt) if which == "q" else (wkt, xk[h], bk_sb, kt)
                )
                p = ps.tile([128, 512], F32, name=f"p{which}{t}{h}", tag="proj", bufs=2)
                for dp in range(ND // 2):
                    nc.tensor.matmul(
                        p,
                        qk_pair(w_t, dp, t),
                        x_pair(x_h, dp),
                        start=(dp == 0),
                        stop=(dp == ND // 2 - 1),
                        perf_mode=DR,
                    )

                def evac():
                    nc.vector.tensor_scalar(
                        dst[t][:, 512 * h : 512 * (h + 1)],
                        p,
                        bias[:, t : t + 1],
                        None,
                        ALU.add,
                    )
                    done.add((which, t, h))

                if defer:
                    evac_q.append(evac)
                else:
                    evac()

            def proj_v(sb):
                """v projection for key block sb -> va[sb] [128 keys, 512 slots].
                Matmul runs on the compact 272-col layout (17-col head slots);
                the evac spreads slots to 32-col alignment for ctx col-tiling."""
                p = ps.tile([128, 512], F32, name=f"pv{sb}", tag="proj", bufs=2)
                h, q = divmod(sb, 4)
                for d in range(ND):
                    nc.tensor.matmul(
                        p[:, 0:272],
                        xv_v(d, h, q),
                        wv_v(d),
                        start=(d == 0),
                        stop=(d == ND - 1),
                    )

                def evac():
                    va_v = va[sb][:].rearrange("p (a b) -> p a b", b=32)[:, :, 0:17]
                    p_v = p[:, 0:272].rearrange("p (a b) -> p a b", b=17)
                    bb_v = biasB[:].rearrange("p (a b) -> p a b", b=17)
                    nc.vector.tensor_add(va_v, p_v, bb_v)
                    done.add(("va", sb))

                evac_q.append(evac)

            og_tiles = {}

            def po_group(n, mt, dc, tail=False):
                """output projection for token block (n, mt), d-half dc."""
                p = ps.tile([128, 512], F32, name=f"po{n}{mt}{dc}", tag="proj", bufs=2)
                for c in range(4):
                    nc.tensor.matmul(
                        p,
                        ctxp[c][:, 512 * n + 128 * mt : 512 * n + 128 * (mt + 1)],
                        wot_v(c, dc),
                        start=(c == 0),
                        stop=(c == 3),
                    )
                if (n, mt) not in og_tiles:
                    og_tiles[(n, mt)] = ap_.tile(
                        [128, 1024], BF16, name=f"og{n}{mt}", tag="og", bufs=3
                    )
                og = og_tiles[(n, mt)]

                def evac():
                    # ACT evacuates (it has slack; DVE runs the psB exp
                    # stream); in the tail split between both
                    if tail and dc == 1:
                        nc.vector.tensor_copy(og[:, 512 * dc : 512 * (dc + 1)], p)
                    else:
                        nc.scalar.copy(og[:, 512 * dc : 512 * (dc + 1)], p)
                    if dc == 1:
                        r0 = 512 * n + 128 * mt
                        eng = (nc.gpsimd, nc.sync, nc.scalar)[mt % 3] if tail else (
                            nc.gpsimd if mt % 2 == 0 else nc.sync
                        )
                        eng.dma_start(out=out_d[r0 : r0 + 128, :], in_=og)

                evac_q.append(evac)

            # ============ lead-in: first projections ====
            # (emitted before biasB: the PE queue is FIFO, and biasB waits on
            # a slow SWDGE const DMA -- it must not block these)
            proj_qk("k", 0, 0, defer=False)
            proj_qk("q", 0, 0, defer=False)
            # biasB (va bias broadcast) on the proj ring
            biasB_ps = ps.tile([128, 512], F32, name="biasB_ps", tag="proj", bufs=2)
            nc.tensor.matmul(biasB_ps[:, 0:272], ones1, bvrow_sb, start=True, stop=True)
            nc.vector.tensor_copy(biasB, biasB_ps[:, 0:272])

            # ============ fill schedule (deadline order, >=4 chunks slack) ==
            fill = []
            fill.append(lambda: proj_qk("k", 0, 1))      # c2,  need c6
            fill.append(lambda: proj_qk("k", 1, 0))      # c4,  need c12
            fill.append(lambda: proj_qk("q", 1, 0))      # c6,  need c12
            fill.append(lambda: proj_qk("k", 1, 1))      # c8,  need c19
            fill.append(lambda: proj_qk("k", 2, 0))      # c10, need c25
            fill.append(lambda: proj_qk("q", 2, 0))      # c12, need c25
            fill.append(lambda: proj_v(0))               # c14
            fill.append(lambda: proj_v(1))               # c16
            fill.append(lambda: proj_v(2))               # c18
            fill.append(lambda: proj_v(3))               # c20
            fill.append(lambda: proj_qk("k", 2, 1))      # c22, need c32
            fill.append(lambda: proj_qk("k", 3, 0))      # c24, need c38
            fill.append(lambda: proj_qk("q", 3, 0))      # c26, need c38
            fill.append(lambda: proj_v(4))               # c28
            fill.append(lambda: proj_v(5))               # c30
            fill.append(lambda: proj_v(6))               # c32
            fill.append(lambda: proj_v(7))               # c34
            fill.append(lambda: proj_qk("k", 3, 1))      # c36, need c44
            for t in range(NQ):
                fill.append(lambda t=t: proj_qk("q", t, 1))  # c38.., need c51+
            fill = list(reversed(fill))  # pop() from the end

            # ============ attention ============
            SLICES = [
                (n, t, i, j)
                for n in range(2)
                for t in range(NQ)
                for i in range(NKB)
                for j in range(NJ)
            ]
            LAG = 2              # chunks of delay before n1 ctx mms hit the PE fifo
            backlog = deque()    # completed chunks awaiting inline ctx emission
            chunk_tiles = {}     # chunk -> (ps tile, ex tile, width)
            chunk_members = {}   # chunk -> list of (n,t,i,j,pos)
            ctx_ps = {}          # (n,t) -> psum tile
            blocks_closed = [0]  # count of (n,t) blocks finalized
            tail_inline = [False]
            pend = deque()       # ready groups: (t, i, [(j, ext, pos), ...])
            pend_build = {}      # (t,i) -> partial member list
            pend_done = [0] * NQ # groups emitted per n0 block

            def emit_ctx(n, t, i, j, ex_t, pos):
                assert ("va", i) in done, f"ctx({n},{t},{i},{j}) before proj_v({i})"
                if (n, t) not in ctx_ps:
                    ctx_ps[(n, t)] = ps.tile(
                        [128, 512], F32, name=f"ctx{n}{t}", tag="ctx", bufs=1
                    )
                m = NJ * t + j
                nc.tensor.matmul(
                    ctx_ps[(n, t)][32 * j : 32 * (j + 1), :],
                    va[i][:, 32 * m : 32 * (m + 1)],
                    ex_t[:, 512 * pos : 512 * (pos + 1)],
                    start=(i == 0),
                    stop=(i == NKB - 1),
                    tile_position=(0, 32 * j),
                    skip_group_check=True,
                )

            # Softmax-chain pipeline: every link runs 2 chunks after its
            # producer, on its own engine, so no engine queue ever
            # head-of-line blocks waiting for another engine:
            #   +2 stage evac (ACT) . +4 den sel-matmul (PE) .
            #   +6 reciprocal (DVE) . +8 rbw broadcast (PE) + normalize (DVE)
            sched = []
            cur_chunk = [0]

            def at_chunk(k, fn):
                sched.append((cur_chunk[0] + k, fn))

            def pump_sched(c):
                ready = [x for x in sched if x[0] <= c]
                if ready:
                    sched[:] = [x for x in sched if x[0] > c]
                    for _, fn in ready:
                        fn()

            def t_end(n, t, inline=False):
                cps = ctx_ps.pop((n, t))

                def link(k, fn):
                    if inline:
                        fn()
                    else:
                        at_chunk(k, fn)

                def s_stage():
                    stage = ap_.tile(
                        [128, 512], F32R, name=f"st{n}{t}", tag="stage", bufs=2
                    )
                    # ACT does the evacuation: DVE is loaded with the psB exp
                    # stream and a DVE burst here would stall it
                    nc.scalar.copy(stage, cps)

                    def s_den():
                        den_ps = ps.tile(
                            [128, 512], F32, name=f"dp{n}{t}", tag="proj", bufs=2
                        )
                        nc.tensor.matmul(
                            den_ps[0:4, :], sel_sb, stage, start=True, stop=True
                        )

                        def s_recip():
                            with tc.high_priority(offset=-160):
                                recip = ap_.tile(
                                    [4, 512], F32, name=f"rc{n}{t}", tag="recip",
                                    bufs=2,
                                )
                                scratch = ap_.tile(
                                    [4, 512], F32, name=f"rs{n}{t}", tag="recip",
                                    bufs=2,
                                )
                                nc.vector.reciprocal_approx_accurate(
                                    recip, den_ps[0:4, :], scratch
                                )
                                recipr = ap_.tile(
                                    [4, 512], F32R, name=f"rr{n}{t}", tag="recipr",
                                    bufs=2,
                                )
                                nc.vector.tensor_copy(recipr, recip)

                            def s_fin():
                                rbw = ps.tile(
                                    [128, 512], F32, name=f"rb{n}{t}", tag="proj",
                                    bufs=2,
                                )
                                nc.tensor.matmul(
                                    rbw, ind_sb, recipr, start=True, stop=True
                                )
                                # full-width normalize: rbw pad/den rows are 0
                                # (ind has no ones there) so pad rows of ctxp
                                # become 0, and wot's matching rows are 0.
                                nc.vector.scalar_tensor_tensor(
                                    ctxp[t][:, 512 * n : 512 * (n + 1)],
                                    rbw,
                                    1.0,
                                    stage[:].bitcast(F32),
                                    ALU.mult,
                                    ALU.mult,
                                )

                            link(2, s_fin)

                        link(2, s_recip)

                    link(2, s_den)

                link(2, s_stage)

            def close_block():
                b = blocks_closed[0]
                bn, bt = divmod(b, NQ)
                t_end(bn, bt, inline=tail_inline[0])
                blocks_closed[0] += 1
                if (bn, bt) == (0, NQ - 1):
                    # n0 ctxp done: queue n0 output projection (pops after
                    # the remaining pre-queued fill items)
                    po_items = [
                        (lambda mt=mt, dc=dc: po_group(0, mt, dc))
                        for mt in range(4)
                        for dc in range(2)
                    ]
                    fill[:0] = po_items[::-1]
                    # no-op boundary slots so the pipelined (0,3) softmax
                    # chain (+8 chunks) lands before the first po(0) read
                    fill.extend([lambda: None] * 4)

            def drain_pend(maxn):
                """Emit up to maxn deferred n0 ctx groups (strict queue order;
                the head blocks until its va block's projection is emitted)."""
                while maxn > 0 and pend:
                    t2, i2, members = pend[0]
                    if ("va", i2) not in done:
                        break
                    assert blocks_closed[0] == t2, (
                        f"pend drain block {t2} but closed {blocks_closed[0]}"
                    )
                    pend.popleft()
                    for (j2, ext2, p2) in members:
                        emit_ctx(0, t2, i2, j2, ext2, p2)
                    pend_done[t2] += 1
                    if pend_done[t2] == NKB:
                        close_block()
                    maxn -= 1

            def process_ctx_batch(members_ext):
                members, ext = members_ext
                for (n2, t2, i2, j2, p2) in members:
                    if n2 == 0:
                        g = pend_build.setdefault((t2, i2), [])
                        g.append((j2, ext, p2))
                        if len(g) == NJ:
                            pend.append((t2, i2, pend_build.pop((t2, i2))))
                        continue
                    b2 = n2 * NQ + t2
                    while blocks_closed[0] < b2:
                        if blocks_closed[0] < NQ:
                            before = blocks_closed[0]
                            drain_pend(10**9)
                            assert blocks_closed[0] > before, "pend drain stuck"
                        else:
                            close_block()
                    emit_ctx(n2, t2, i2, j2, ext, p2)

            def pace(c):
                # every other chunk while projection fills remain; from c70
                # (po-era) every 4th, spreading real PE work deeper into the
                # stream so the HAM clock-gate stays open into the tail
                if c < 70:
                    return c >= 2 and c % 2 == 0
                return c % 4 == 0

            for s, (n, t, i, j) in enumerate(SLICES):
                c, pos, size = chunk_of(s)
                cur_chunk[0] = c
                width = min(size, NSL - (s - pos))
                if pos == 0:
                    pst = psA if c % 2 == 0 else psB
                    tag = "exA" if c % 2 == 0 else "exB"
                    ext = ap_.tile(
                        [128, 512 * width], BF16, name=f"ex{c}",
                        tag=f"{tag}{width}", bufs=10,
                    )
                    chunk_tiles[c] = (pst, ext, width)
                    chunk_members[c] = []
                pst, ext, width = chunk_tiles[c]
                chunk_members[c].append((n, t, i, j, pos))
                assert ("k", t, i // 4) in done, f"scores({n},{t},{i}) before kt"
                assert ("q", t, n) in done, f"scores({n},{t},{i}) before qt"
                # scores matmul into the chunk's psum slice
                nc.tensor.matmul(
                    pst[:, 512 * pos : 512 * (pos + 1)],
                    kt[t][32 * j : 32 * (j + 1), 128 * i : 128 * (i + 1)],
                    qt[t][32 * j : 32 * (j + 1), 512 * n : 512 * (n + 1)],
                    start=True,
                    stop=True,
                    tile_position=(32 * j, 0),
                )
                if pos == width - 1:
                    # chunk complete: exp it (ACT spline / DVE Schraudolph at
                    # elevated priority so neither stream stalls its ring);
                    # emit lagged ctx mms; drain the deferred-n0 queue; pace fill
                    with tc.high_priority(offset=-80):
                        if dve_chunk(c):
                            nc.vector.tensor_scalar(
                                ext[:].bitcast(I16),
                                pst[:, 0 : 512 * width],
                                A16,
                                B16,
                                ALU.mult,
                                ALU.add,
                            )
                        else:
                            nc.scalar.activation(
                                ext, pst[:, 0 : 512 * width], AF.Exp, scale=SCALE
                            )
                    backlog.append((chunk_members.pop(c), ext))
                    lag_now = LAG if c < 100 else 0
                    while len(backlog) > lag_now:
                        process_ctx_batch(backlog.popleft())
                    pump_sched(c)
                    pump_evacs()
                    # keep-warm: a weight-load touches the PE array every
                    # chunk with NO psum write, so it couples to nothing
                    # but keeps the HAM activity window from reading idle
                    nc.tensor.ldweights(weights=wub[:, 0:128])
                    do_fill = bool(fill) and pace(c)
                    drain_pend(1 if do_fill else 2)
                    if do_fill:
                        fill.pop()()


            # tail: drain ctx, close the last blocks inline, po(1) + flush
            while backlog:
                process_ctx_batch(backlog.popleft())
            drain_pend(10**9)
            # the last block(s) close here: run their softmax chain inline
            # (sel-matmul den gather -- the PE is idle in the tail)
            tail_inline[0] = True
            while blocks_closed[0] < 2 * NQ:
                close_block()
            while sched:
                due, fn = sched.pop(0)
                fn()
            pump_evacs()
            while fill:
                fill.pop()()
            pump_evacs()
            # n1 output projection: pipeline groups so each evacuation trails
            # the next group's matmuls by one slot
            for k, (mt, dc) in enumerate([(m, d) for m in range(4) for d in range(2)]):
                po_group(1, mt, dc, tail=True)
                nc.tensor.ldweights(weights=wub[:, 0:128])
                while len(evac_q) > 1:
                    evac_q.popleft()()
            pump_evacs()

    nc.finalize()
    return nc


def prep_core_weights(g, Wq, bq, Wk, bk, Wv, bv, Wo):
    C0 = 256 * g
    wqt = np.zeros((D, 512), np.float32)
    wkt = np.zeros((D, 512), np.float32)
    wvt = np.zeros((D, 272), np.float32)
    bvrow = np.zeros((1, 272), np.float32)
    bqp = np.zeros((128, NQ), np.float32)
    bkp = np.zeros((128, NQ), np.float32)
    for t in range(NQ):
        for j in range(NJ):
            src = C0 + 64 * t + 16 * j
            wqt[:, 128 * t + 32 * j : 128 * t + 32 * j + E] = Wq[src : src + E, :].T
            wkt[:, 128 * t + 32 * j : 128 * t + 32 * j + E] = Wk[src : src + E, :].T
            m = NJ * t + j
            wvt[:, 17 * m : 17 * m + E] = Wv[src : src + E, :].T
            bvrow[0, 17 * m : 17 * m + E] = bv[src : src + E]
            bvrow[0, 17 * m + E] = 1.0
            bqp[32 * j : 32 * j + E, t] = bq[src : src + E]
            bkp[32 * j : 32 * j + E, t] = bk[src : src + E]
    wot = np.zeros((512, D), np.float32)
    for t in range(NQ):
        for j in range(NJ):
            src = C0 + 64 * t + 16 * j
            wot[128 * t + 32 * j : 128 * t + 32 * j + E, :] = Wo[:, src : src + E].T
    ind = np.zeros((4, 128), np.float32)
    sel = np.zeros((128, 4), np.float32)
    for j in range(NJ):
        ind[j, 32 * j : 32 * j + E] = 1.0
        sel[32 * j + E, j] = 1.0
    return {
        "sel": round_fp32r(sel),
        "wqt": pack_w_tmaj(fp8_np(wqt)),
        "wkt": pack_w_tmaj(fp8_np(wkt)),
        "wvt": pack_dmaj(bf16_np(wvt)),
        "wot": pack_dmaj(bf16_np(wot)),
        "bvrow": bvrow,
        "bqp": bqp,
        "bkp": bkp,
        "ind": round_fp32r(ind),
    }


def pack_dmaj(x):
    """[(a*128), c] -> [128, a*c]: partition-major with d-chunk-contiguous cols."""
    a = x.shape[0] // 128
    c = x.shape[1]
    return np.ascontiguousarray(
        x.reshape(a, 128, c).transpose(1, 0, 2).reshape(128, a * c)
    )


def pack_w_tmaj(w):
    """[(d*128), (t*128)] -> [128, t*d*128]: t-quad outer, d-chunk inner."""
    a = w.reshape(8, 128, 4, 128)
    return np.ascontiguousarray(a.transpose(1, 2, 0, 3).reshape(128, 4096))


def pack_x(x):
    """[(d*128), (h*512)] -> [128, h*d*512]: h-half outer, d-chunk inner."""
    return np.ascontiguousarray(
        x.reshape(8, 128, 2, 512).transpose(1, 2, 0, 3).reshape(128, 8192)
    )


def prep_in_maps(Q, K, V, Wq, bq, Wk, bk, Wv, bv, Wo):
    group_w = [prep_core_weights(g, Wq, bq, Wk, bk, Wv, bv, Wo) for g in range(4)]
    xt = []
    for b in range(B):
        xt.append(
            {
                "xq": pack_x(fp8_np(Q[b].T)),
                "xk": pack_x(fp8_np(K[b].T)),
                "xv": pack_x(bf16_np(V[b].T)),
            }
        )
    in_maps = []
    for c in range(8):
        b, g = c // 4, c % 4
        m = dict(group_w[g])
        m.update(xt[b])
        in_maps.append(m)
    return in_maps


def assemble_output(results, bo):
    out = np.zeros((B, S, D), np.float32)
    for b in range(B):
        acc = np.zeros((S, D), np.float64)
        for g in range(4):
            acc += results[4 * b + g]["out_part"].astype(np.float64)
        out[b] = (acc + bo.astype(np.float64)).astype(np.float32)
    return out


_NC_CACHE = {}


def _get_nc():
    if "nc" not in _NC_CACHE:
        _NC_CACHE["nc"] = build_nc()
    return _NC_CACHE["nc"]


def kernel(Q, K, V, Wq, bq, Wk, bk, Wv, bv, Wo, bo):
    import time

    from concourse.bass_utils import run_bass_kernel_spmd

    nc = _get_nc()
    in_maps = prep_in_maps(
        np.asarray(Q, np.float32),
        np.asarray(K, np.float32),
        np.asarray(V, np.float32),
        np.asarray(Wq, np.float32),
        np.asarray(bq, np.float32),
        np.asarray(Wk, np.float32),
        np.asarray(bk, np.float32),
        np.asarray(Wv, np.float32),
        np.asarray(bv, np.float32),
        np.asarray(Wo, np.float32),
    )
    # Retries: a first execution after NEFF load occasionally hits a
    # transient NRT_EXEC_UNIT_UNRECOVERABLE; re-running recovers.
    last = None
    for attempt in range(3):
        try:
            res = run_bass_kernel_spmd(nc, in_maps, list(range(8)))
            return assemble_output(res.results, np.asarray(bo, np.float32))
        except Exception as e:
            last = e
            time.sleep(3)
    raise last


# revision 57
# speedup vs baseline: 1.0028x; 1.0028x over previous
"""Self-contained TRN2 Bass kernel for nn_MultiHeadAttention_77833397338481.

kernel(**inputs) takes the FULL unsharded inputs (Q, K, V [2,1024,1024],
Wq/Wk/Wv/Wo [1024,1024], biases [1024]) and returns the FULL output
[2, 1024, 1024]. 8 NeuronCores = batch(2) x head-group(4).

Design: the softmax exp stream (16.8M exps/core) is split between the
Scalar engine (ACT spline exp, the 1536-col psA chunks = 60%) and the
Vector engine (Schraudolph bit-trick exp for the 1024-col psB chunks:
one tensor_scalar -> round-to-int16 bitcast as bf16, constant tuned
end-to-end so mixed exact/approx softmax rows stay unbiased). The two
2-deep PSUM rings then never wait on each other's engine.
 - bf16 matmul operands, fp32 PSUM accumulation; fp8 DoubleRow q/k
   projections (2 d-chunks per matmul).
 - ctx matmuls 4-way column-tiled (tile_position=(0,32j)); V padded to
   32-col head slots with a ones column producing softmax denominators
   in the same matmuls.
 - every PSUM->SBUF evacuation is emitted one fill slot (~2 chunks)
   after its producing matmuls (evac_q), and the softmax normalize
   chain is a 4-link pipeline stepping 2 chunks per link (stage evac on
   ACT -> den sel-matmul on PE -> reciprocal on DVE -> rbw broadcast on
   PE + full-width normalize on DVE), so no engine queue head-of-line
   blocks waiting on another engine.
 - inputs are host-prepacked to SBUF layout; the critical first-chunk
   set (t0 weight slices + first token halves, 1.28MB) rides alone at
   the heads of the two fast HWDGE rings (HBM bandwidth is the lead-in
   limiter and rings are FIFO); first exp fires ~20us in.
 - projections + n0 output projection are paced "fill" work inside the
   attention phase; n1 output projection forms the tail with [128,1024]
   output DMAs split across rings; ldweights keep-alives nudge the HAM
   clock gate without touching PSUM.
"""

import math
from collections import deque

import numpy as np
import ml_dtypes

import concourse.bacc as bacc
import concourse.mybir as mybir
import concourse.tile as tile

F32 = mybir.dt.float32
FP8 = mybir.dt.float8e4
F32R = mybir.dt.float32r
BF16 = mybir.dt.bfloat16
I16 = mybir.dt.int16
AF = mybir.ActivationFunctionType
ALU = mybir.AluOpType

D = 1024
S = 1024
B = 2
E = 16
NQ = 4          # quads per core
NJ = 4          # heads per quad
ND = 8          # 128-row d chunks
NKB = 8         # 128-key blocks
SCALE = 1.0 / 32.0
NSL = 2 * NQ * NKB * NJ   # 256 score slices of [128 keys, 512 queries]

# --- DVE Schraudolph exp constants (bf16-bitcast domain) ---
# i16 = round(x * A16 + B16); bitcast bf16 ~= exp(SCALE*x) * (1 + eps(frac)),
# C_MAGIC chosen so E[ln(1+f-c) - f ln2] = 0 over f~U[0,1): eps is zero-mean.
C_MAGIC = 5.0
A16 = SCALE * math.log2(math.e) * 128.0
B16 = 127 * 128.0 - C_MAGIC
# exp engine split: ACT owns the psA ring (even chunks, 1536 cols = 60%),
# DVE owns the psB ring (odd chunks, 1024 cols = 40%) — the two 2-deep
# PSUM rings then never wait on each other's engine.
def dve_chunk(c):
    return c % 2 == 1


def bf16_np(x):
    return np.ascontiguousarray(x, np.float32).astype(ml_dtypes.bfloat16)


def fp8_np(x):
    return np.ascontiguousarray(x, np.float32).astype(ml_dtypes.float8_e4m3)


def round_fp32r(x):
    u = np.ascontiguousarray(x, np.float32).view(np.uint32)
    r = ((u.astype(np.uint64) + 0x800) & 0xFFFFF000).astype(np.uint32)
    return r.view(np.float32)


def chunk_of(s):
    """Global slice s -> (chunk id, position, nominal size). A chunks (even)
    hold 3 slices, B chunks (odd) hold 2."""
    pair, w = divmod(s, 5)
    if w < 3:
        return 2 * pair, w, 3
    return 2 * pair + 1, w - 3, 2


def build_nc():
    nc = bacc.Bacc("TRN2", target_bir_lowering=False, debug=False, num_devices=8)

    # inputs are host-prepacked into the exact SBUF layouts (partition-major,
    # d-chunk-contiguous) so every load is one contiguous [128, N] DMA
    xq_d = nc.dram_tensor("xq", [128, 8192], FP8, kind="ExternalInput")
    xk_d = nc.dram_tensor("xk", [128, 8192], FP8, kind="ExternalInput")
    xv_d = nc.dram_tensor("xv", [128, 8192], BF16, kind="ExternalInput")
    wqt_d = nc.dram_tensor("wqt", [128, 4096], FP8, kind="ExternalInput")
    wkt_d = nc.dram_tensor("wkt", [128, 4096], FP8, kind="ExternalInput")
    wvt_d = nc.dram_tensor("wvt", [128, 2176], BF16, kind="ExternalInput")
    wot_d = nc.dram_tensor("wot", [128, 4096], BF16, kind="ExternalInput")
    bvrow_d = nc.dram_tensor("bvrow", [1, 272], F32, kind="ExternalInput")
    bqp_d = nc.dram_tensor("bqp", [128, NQ], F32, kind="ExternalInput")
    bkp_d = nc.dram_tensor("bkp", [128, NQ], F32, kind="ExternalInput")
    ind_d = nc.dram_tensor("ind", [4, 128], F32R, kind="ExternalInput")
    sel_d = nc.dram_tensor("sel", [128, 4], F32R, kind="ExternalInput")
    out_d = nc.dram_tensor("out_part", [S, D], BF16, kind="ExternalOutput")

    with tile.TileContext(nc) as tc:
        with (
            tc.tile_pool(name="persist", bufs=1) as pp,
            tc.tile_pool(name="attn", bufs=1) as ap_,
            tc.tile_pool(name="psum", space="PSUM", bufs=1) as ps,
        ):
            # --- warm-up data (no DMA dependency): PE burst from ~0.4us so the
            # HAM clock gate opens before the first projections ---
            wub = pp.tile([128, 512], BF16, name="wub")
            nc.vector.memset(wub, 1.0)
            ones1 = pp.tile([1, 128], F32, name="ones1")
            nc.vector.memset(ones1, 1.0)
            wu = ps.tile([128, 512], F32, name="wu", tag="proj", bufs=2)
            for _ in range(7):
                nc.tensor.matmul(wu, wub[:, 0:128], wub, start=True, stop=True)

            # --- batched input DMAs: one per tensor(-half), d-major layout,
            # split across the three DGE rings (gpsimd=SWDGE k/v path,
            # sync=SP-HWDGE q path, scalar=ACT-HWDGE weights) in
            # deadline-priority order ---
            wkt = pp.tile([128, 8 * 512], FP8, name="wkt")
            wqt = pp.tile([128, 8 * 512], FP8, name="wqt")
            wvt = pp.tile([128, 8 * 272], BF16, name="wvt")
            wot_sb = pp.tile([128, 4 * 1024], BF16, name="wot")
            xk = [pp.tile([128, 8 * 512], FP8, name=f"xk{h}") for h in range(2)]
            xq = [pp.tile([128, 8 * 512], FP8, name=f"xq{h}") for h in range(2)]
            xv = [pp.tile([128, 8 * 512], BF16, name=f"xv{h}") for h in range(2)]

            # HBM bandwidth (~358 GB/s) is the lead-in limiter, and each DGE
            # ring is FIFO: put ONLY the first-chunk critical set (wkt/xk0,
            # then wqt/xq0) at the heads of the two HWDGE rings so it gets the
            # full bandwidth; everything later queues behind ON THE SAME rings.
            # The slow-starting SWDGE ring gets only tiny late-need consts.
            bq_sb = pp.tile([128, NQ], F32, name="bq_sb")
            bk_sb = pp.tile([128, NQ], F32, name="bk_sb")
            bvrow_sb = pp.tile([1, 272], F32, name="bvrow_sb")
            ind_sb = pp.tile([4, 128], F32R, name="ind_sb")
            sel_sb = pp.tile([128, 4], F32R, name="sel_sb")
            # W tensors are t-major so the first quad's weights (128KB each)
            # arrive in ~1us; the critical first-chunk set is then just
            # wkt_t0+xk0 / wqt_t0+xq0
            # critical first-chunk set (1.28MB) alone on the two fast HWDGE
            # rings; everything else rides the slow-starting SWDGE ring or
            # queues behind on sync, in deadline order
            nc.scalar.dma_start(out=wkt[:, 0:1024], in_=wkt_d[:, 0:1024])
            nc.scalar.dma_start(out=wqt[:, 0:1024], in_=wqt_d[:, 0:1024])
            nc.sync.dma_start(out=bq_sb, in_=bqp_d[:])
            nc.sync.dma_start(out=bk_sb, in_=bkp_d[:])
            nc.sync.dma_start(out=xk[0], in_=xk_d[:, 0:4096])
            nc.sync.dma_start(out=xq[0], in_=xq_d[:, 0:4096])
            nc.scalar.dma_start(out=wkt[:, 1024:4096], in_=wkt_d[:, 1024:4096])
            nc.scalar.dma_start(out=wqt[:, 1024:4096], in_=wqt_d[:, 1024:4096])
            nc.gpsimd.dma_start(out=wvt, in_=wvt_d[:])
            nc.gpsimd.dma_start(out=ind_sb, in_=ind_d[:])
            nc.gpsimd.dma_start(out=sel_sb, in_=sel_d[:])
            nc.gpsimd.dma_start(out=bvrow_sb, in_=bvrow_d[:])
            nc.sync.dma_start(out=xk[1], in_=xk_d[:, 4096:8192])
            nc.sync.dma_start(out=xv[0], in_=xv_d[:, 0:4096])
            nc.sync.dma_start(out=xv[1], in_=xv_d[:, 4096:8192])
            nc.sync.dma_start(out=xq[1], in_=xq_d[:, 4096:8192])
            nc.gpsimd.dma_start(out=wot_sb, in_=wot_d[:])
            # pre-load the ACT exp table during the lead-in (after the DMA
            # issues -- the ~1.3us table load must not delay them)
            actw = ap_.tile([1, 128], BF16, name="actw", tag="actw", bufs=1)
            nc.scalar.activation(actw, ones1, AF.Exp, scale=SCALE)

            # slice helpers
            def xk_v(d, h):
                return xk[h][:, 512 * d : 512 * (d + 1)]

            def xq_v(d, h):
                return xq[h][:, 512 * d : 512 * (d + 1)]

            def xv_v(d, h, q):
                return xv[h][:, 512 * d + 128 * q : 512 * d + 128 * (q + 1)]

            def wk_v(d, t):
                return wkt[:, 1024 * t + 128 * d : 1024 * t + 128 * (d + 1)]

            def wq_v(d, t):
                return wqt[:, 1024 * t + 128 * d : 1024 * t + 128 * (d + 1)]

            def wv_v(d):
                return wvt[:, 272 * d : 272 * (d + 1)]

            def wot_v(c, dc):
                return wot_sb[:, 1024 * c + 512 * dc : 1024 * c + 512 * (dc + 1)]

            # --- persistent activations ---
            qt = [pp.tile([128, S], BF16, name=f"qt{t}") for t in range(NQ)]
            kt = [pp.tile([128, S], BF16, name=f"kt{t}") for t in range(NQ)]
            va = [pp.tile([128, 512], BF16, name=f"va{s}") for s in range(NKB)]
            for sb in range(NKB):
                # gpsimd memsets: the DVE must be free for the first evacs
                nc.gpsimd.memset(va[sb], 0.0)
            ctxp = [pp.tile([128, S], BF16, name=f"ctxp{c}") for c in range(4)]

            # --- psum ring for scores/exp (5 banks) ---
            psA = ps.tile([128, 1536], F32, name="psA", tag="psA", bufs=1)
            psB = ps.tile([128, 1024], F32, name="psB", tag="psB", bufs=1)

            biasB = pp.tile([128, 272], F32, name="biasB")

            # emitted-producer tracking: a consumer emitted before its
            # producer would silently read stale/garbage SBUF on hardware
            done = set()

            # Deferred-evacuation queue: a PSUM->SBUF evacuation emitted right
            # after its producing matmuls would sit at the head of the ACT/DVE
            # queue waiting on the PE and stall the exp stream behind it.
            # Emitters push their evac closure here; it is emitted one fill
            # slot (~2 chunks) later, when the producer has already executed.
            evac_q = deque()

            def pump_evacs():
                while evac_q:
                    evac_q.popleft()()

            # ============ projection emitters ============
            DR = mybir.MatmulPerfMode.DoubleRow

            def qk_pair(w_t, dp, t):
                """[K=128, Ko=2, M=128] DoubleRow weight AP for d-pair dp
                (t-major layout: the d-pair is 256 contiguous cols)."""
                v = w_t[:, 1024 * t + 256 * dp : 1024 * t + 256 * (dp + 1)]
                return v.rearrange("p (i c) -> p i c", i=2)

            def x_pair(x_h, dp):
                """[K=128, Ko=2, N=512] DoubleRow moving AP for d-pair dp."""
                v = x_h[:, 1024 * dp : 1024 * (dp + 1)]
                return v.rearrange("p (i c) -> p i c", i=2)

            def proj_qk(which, t, h, defer=True):
                """q/k projection for quad t, s-half h -> qt/kt[t][:, 512h:].
                fp8 DoubleRow: 4 matmuls of 2 d-chunks each."""
                w_t, x_h, bias, dst = (
                    (wqt, xq[h], bq_sb, qt) if which == "q" else (wkt, xk[h], bk_sb, kt)
                )
                p = ps.tile([128, 512], F32, name=f"p{which}{t}{h}", tag="proj", bufs=2)
                for dp in range(ND // 2):
                    nc.tensor.matmul(
                        p,
                        qk_pair(w_t, dp, t),
                        x_pair(x_h, dp),
                        start=(dp == 0),
                        stop=(dp == ND // 2 - 1),
                        perf_mode=DR,
                    )

                def evac():
                    nc.vector.tensor_scalar(
                        dst[t][:, 512 * h : 512 * (h + 1)],
                        p,
                        bias[:, t : t + 1],
                        None,
                        ALU.add,
                    )
                    done.add((which, t, h))

                if defer:
                    evac_q.append(evac)
                else:
                    evac()

            def proj_v(sb):
                """v projection for key block sb -> va[sb] [128 keys, 512 slots].
                Matmul runs on the compact 272-col layout (17-col head slots);
                the evac spreads slots to 32-col alignment for ctx col-tiling."""
                p = ps.tile([128, 512], F32, name=f"pv{sb}", tag="proj", bufs=2)
                h, q = divmod(sb, 4)
                for d in range(ND):
                    nc.tensor.matmul(
                        p[:, 0:272],
                        xv_v(d, h, q),
                        wv_v(d),
                        start=(d == 0),
                        stop=(d == ND - 1),
                    )

                def evac():
                    va_v = va[sb][:].rearrange("p (a b) -> p a b", b=32)[:, :, 0:17]
                    p_v = p[:, 0:272].rearrange("p (a b) -> p a b", b=17)
                    bb_v = biasB[:].rearrange("p (a b) -> p a b", b=17)
                    nc.vector.tensor_add(va_v, p_v, bb_v)
                    done.add(("va", sb))

                evac_q.append(evac)

            og_tiles = {}

            def po_group(n, mt, dc, tail=False):
                """output projection for token block (n, mt), d-half dc."""
                p = ps.tile([128, 512], F32, name=f"po{n}{mt}{dc}", tag="proj", bufs=2)
                for c in range(4):
                    nc.tensor.matmul(
                        p,
                        ctxp[c][:, 512 * n + 128 * mt : 512 * n + 128 * (mt + 1)],
                        wot_v(c, dc),
                        start=(c == 0),
                        stop=(c == 3),
                    )
                if (n, mt) not in og_tiles:
                    og_tiles[(n, mt)] = ap_.tile(
                        [128, 1024], BF16, name=f"og{n}{mt}", tag="og", bufs=3
                    )
                og = og_tiles[(n, mt)]

                def evac():
                    # ACT evacuates (it has slack; DVE runs the psB exp
                    # stream); in the tail split between both
                    if tail and dc == 1:
                        nc.vector.tensor_copy(og[:, 512 * dc : 512 * (dc + 1)], p)
                    else:
                        nc.scalar.copy(og[:, 512 * dc : 512 * (dc + 1)], p)
                    if dc == 1:
                        r0 = 512 * n + 128 * mt
                        eng = (nc.gpsimd, nc.sync, nc.scalar)[mt % 3] if tail else (
                            nc.gpsimd if mt % 2 == 0 else nc.sync
                        )
                        eng.dma_start(out=out_d[r0 : r0 + 128, :], in_=og)

                evac_q.append(evac)

            # ============ lead-in: first projections ====
            # (emitted before biasB: the PE queue is FIFO, and biasB waits on
            # a slow SWDGE const DMA -- it must not block these)
            proj_qk("k", 0, 0, defer=False)
            proj_qk("q", 0, 0, defer=False)
            # biasB (va bias broadcast) on the proj ring
            biasB_ps = ps.tile([128, 512], F32, name="biasB_ps", tag="proj", bufs=2)
            nc.tensor.matmul(biasB_ps[:, 0:272], ones1, bvrow_sb, start=True, stop=True)
            nc.vector.tensor_copy(biasB, biasB_ps[:, 0:272])

            # ============ fill schedule (deadline order, >=4 chunks slack) ==
            fill = []
            fill.append(lambda: proj_qk("k", 0, 1))      # c2,  need c6
            fill.append(lambda: proj_qk("k", 1, 0))      # c4,  need c12
            fill.append(lambda: proj_qk("q", 1, 0))      # c6,  need c12
            fill.append(lambda: proj_qk("k", 1, 1))      # c8,  need c19
            fill.append(lambda: proj_qk("k", 2, 0))      # c10, need c25
            fill.append(lambda: proj_qk("q", 2, 0))      # c12, need c25
            fill.append(lambda: proj_v(0))               # c14
            fill.append(lambda: proj_v(1))               # c16
            fill.append(lambda: proj_v(2))               # c18
            fill.append(lambda: proj_v(3))               # c20
            fill.append(lambda: proj_qk("k", 2, 1))      # c22, need c32
            fill.append(lambda: proj_qk("k", 3, 0))      # c24, need c38
            fill.append(lambda: proj_qk("q", 3, 0))      # c26, need c38
            fill.append(lambda: proj_v(4))               # c28
            fill.append(lambda: proj_v(5))               # c30
            fill.append(lambda: proj_v(6))               # c32
            fill.append(lambda: proj_v(7))               # c34
            fill.append(lambda: proj_qk("k", 3, 1))      # c36, need c44
            for t in range(NQ):
                fill.append(lambda t=t: proj_qk("q", t, 1))  # c38.., need c51+
            fill = list(reversed(fill))  # pop() from the end

            # ============ attention ============
            SLICES = [
                (n, t, i, j)
                for n in range(2)
                for t in range(NQ)
                for i in range(NKB)
                for j in range(NJ)
            ]
            LAG = 2              # chunks of delay before n1 ctx mms hit the PE fifo
            backlog = deque()    # completed chunks awaiting inline ctx emission
            chunk_tiles = {}     # chunk -> (ps tile, ex tile, width)
            chunk_members = {}   # chunk -> list of (n,t,i,j,pos)
            ctx_ps = {}          # (n,t) -> psum tile
            blocks_closed = [0]  # count of (n,t) blocks finalized
            tail_inline = [False]
            pend = deque()       # ready groups: (t, i, [(j, ext, pos), ...])
            pend_build = {}      # (t,i) -> partial member list
            pend_done = [0] * NQ # groups emitted per n0 block

            def emit_ctx(n, t, i, j, ex_t, pos):
                assert ("va", i) in done, f"ctx({n},{t},{i},{j}) before proj_v({i})"
                if (n, t) not in ctx_ps:
                    ctx_ps[(n, t)] = ps.tile(
                        [128, 512], F32, name=f"ctx{n}{t}", tag="ctx", bufs=1
                    )
                m = NJ * t + j
                nc.tensor.matmul(
                    ctx_ps[(n, t)][32 * j : 32 * (j + 1), :],
                    va[i][:, 32 * m : 32 * (m + 1)],
                    ex_t[:, 512 * pos : 512 * (pos + 1)],
                    start=(i == 0),
                    stop=(i == NKB - 1),
                    tile_position=(0, 32 * j),
                    skip_group_check=True,
                )

            # Softmax-chain pipeline: every link runs 2 chunks after its
            # producer, on its own engine, so no engine queue ever
            # head-of-line blocks waiting for another engine:
            #   +2 stage evac (ACT) . +4 den sel-matmul (PE) .
            #   +6 reciprocal (DVE) . +8 rbw broadcast (PE) + normalize (DVE)
            sched = []
            cur_chunk = [0]

            def at_chunk(k, fn):
                sched.append((cur_chunk[0] + k, fn))

            def pump_sched(c):
                ready = [x for x in sched if x[0] <= c]
                if ready:
                    sched[:] = [x for x in sched if x[0] > c]
                    for _, fn in ready:
                        fn()

            def t_end(n, t, inline=False):
                cps = ctx_ps.pop((n, t))

                def link(k, fn):
                    if inline:
                        fn()
                    else:
                        at_chunk(k, fn)

                def s_stage():
                    stage = ap_.tile(
                        [128, 512], F32R, name=f"st{n}{t}", tag="stage", bufs=2
                    )
                    # ACT does the evacuation: DVE is loaded with the psB exp
                    # stream and a DVE burst here would stall it
                    nc.scalar.copy(stage, cps)

                    def s_den():
                        den_ps = ps.tile(
                            [128, 512], F32, name=f"dp{n}{t}", tag="proj", bufs=2
                        )
                        nc.tensor.matmul(
                            den_ps[0:4, :], sel_sb, stage, start=True, stop=True
                        )

                        def s_recip():
                            with tc.high_priority(offset=-160):
                                recip = ap_.tile(
                                    [4, 512], F32, name=f"rc{n}{t}", tag="recip",
                                    bufs=2,
                                )
                                scratch = ap_.tile(
                                    [4, 512], F32, name=f"rs{n}{t}", tag="recip",
                                    bufs=2,
                                )
                                nc.vector.reciprocal_approx_accurate(
                                    recip, den_ps[0:4, :], scratch
                                )
                                recipr = ap_.tile(
                                    [4, 512], F32R, name=f"rr{n}{t}", tag="recipr",
                                    bufs=2,
                                )
                                nc.vector.tensor_copy(recipr, recip)

                            def s_fin():
                                rbw = ps.tile(
                                    [128, 512], F32, name=f"rb{n}{t}", tag="proj",
                                    bufs=2,
                                )
                                nc.tensor.matmul(
                                    rbw, ind_sb, recipr, start=True, stop=True
                                )
                                # full-width normalize: rbw pad/den rows are 0
                                # (ind has no ones there) so pad rows of ctxp
                                # become 0, and wot's matching rows are 0.
                                nc.vector.scalar_tensor_tensor(
                                    ctxp[t][:, 512 * n : 512 * (n + 1)],
                                    rbw,
                                    1.0,
                                    stage[:].bitcast(F32),
                                    ALU.mult,
                                    ALU.mult,
                                )

                            link(2, s_fin)

                        link(2, s_recip)

                    link(2, s_den)

                link(2, s_stage)

            def close_block():
                b = blocks_closed[0]
                bn, bt = divmod(b, NQ)
                t_end(bn, bt, inline=tail_inline[0])
                blocks_closed[0] += 1
                if (bn, bt) == (0, NQ - 1):
                    # n0 ctxp done: queue n0 output projection (pops after
                    # the remaining pre-queued fill items)
                    po_items = [
                        (lambda mt=mt, dc=dc: po_group(0, mt, dc))
                        for mt in range(4)
                        for dc in range(2)
                    ]
                    fill[:0] = po_items[::-1]
                    # no-op boundary slots so the pipelined (0,3) softmax
                    # chain (+8 chunks) lands before the first po(0) read
                    fill.extend([lambda: None] * 4)

            def drain_pend(maxn):
                """Emit up to maxn deferred n0 ctx groups (strict queue order;
                the head blocks until its va block's projection is emitted)."""
                while maxn > 0 and pend:
                    t2, i2, members = pend[0]
                    if ("va", i2) not in done:
                        break
                    assert blocks_closed[0] == t2, (
                        f"pend drain block {t2} but closed {blocks_closed[0]}"
                    )
                    pend.popleft()
                    for (j2, ext2, p2) in members:
                        emit_ctx(0, t2, i2, j2, ext2, p2)
                    pend_done[t2] += 1
                    if pend_done[t2] == NKB:
                        close_block()
                    maxn -= 1

            def process_ctx_batch(members_ext):
                members, ext = members_ext
                for (n2, t2, i2, j2, p2) in members:
                    if n2 == 0:
                        g = pend_build.setdefault((t2, i2), [])
                        g.append((j2, ext, p2))
                        if len(g) == NJ:
                            pend.append((t2, i2, pend_build.pop((t2, i2))))
                        continue
                    b2 = n2 * NQ + t2
                    while blocks_closed[0] < b2:
                        if blocks_closed[0] < NQ:
                            before = blocks_closed[0]
                            drain_pend(10**9)
                            assert blocks_closed[0] > before, "pend drain stuck"
                        else:
                            close_block()
                    emit_ctx(n2, t2, i2, j2, ext, p2)

            def pace(c):
                # every other chunk while projection fills remain; from c70
                # (po-era) every 4th, spreading real PE work deeper into the
                # stream so the HAM clock-gate stays open into the tail
                if c < 70:
                    return c >= 2 and c % 2 == 0
                return c % 4 == 0

            for s, (n, t, i, j) in enumerate(SLICES):
                c, pos, size = chunk_of(s)
                cur_chunk[0] = c
                width = min(size, NSL - (s - pos))
                if pos == 0:
                    pst = psA if c % 2 == 0 else psB
                    tag = "exA" if c % 2 == 0 else "exB"
                    ext = ap_.tile(
                        [128, 512 * width], BF16, name=f"ex{c}",
                        tag=f"{tag}{width}", bufs=10,
                    )
                    chunk_tiles[c] = (pst, ext, width)
                    chunk_members[c] = []
                pst, ext, width = chunk_tiles[c]
                chunk_members[c].append((n, t, i, j, pos))
                assert ("k", t, i // 4) in done, f"scores({n},{t},{i}) before kt"
                assert ("q", t, n) in done, f"scores({n},{t},{i}) before qt"
                # scores matmul into the chunk's psum slice
                nc.tensor.matmul(
                    pst[:, 512 * pos : 512 * (pos + 1)],
                    kt[t][32 * j : 32 * (j + 1), 128 * i : 128 * (i + 1)],
                    qt[t][32 * j : 32 * (j + 1), 512 * n : 512 * (n + 1)],
                    start=True,
                    stop=True,
                    tile_position=(32 * j, 0),
                )
                if pos == width - 1:
                    # chunk complete: exp it (ACT spline / DVE Schraudolph at
                    # elevated priority so neither stream stalls its ring);
                    # emit lagged ctx mms; drain the deferred-n0 queue; pace fill
                    with tc.high_priority(offset=-80):
                        if dve_chunk(c):
                            nc.vector.tensor_scalar(
                                ext[:].bitcast(I16),
                                pst[:, 0 : 512 * width],
                                A16,
                                B16,
                                ALU.mult,
                                ALU.add,
                            )
                        else:
                            nc.scalar.activation(
                                ext, pst[:, 0 : 512 * width], AF.Exp, scale=SCALE
                            )
                    backlog.append((chunk_members.pop(c), ext))
                    lag_now = LAG if c < 100 else 0
                    while len(backlog) > lag_now:
                        process_ctx_batch(backlog.popleft())
                    pump_sched(c)
                    pump_evacs()
                    if c % 2 == 0:
                        # keep-warm: a weight-load touches the PE array every
                        # ~2.7us with NO psum write, so it couples to nothing
                        # but keeps the HAM activity window from reading idle
                        nc.tensor.ldweights(weights=wub[:, 0:128])
                    do_fill = bool(fill) and pace(c)
                    drain_pend(1 if do_fill else 2)
                    if do_fill:
                        fill.pop()()


            # tail: drain ctx, close the last blocks inline, po(1) + flush
            while backlog:
                process_ctx_batch(backlog.popleft())
            drain_pend(10**9)
            # the last block(s) close here: run their softmax chain inline
            # (sel-matmul den gather -- the PE is idle in the tail)
            tail_inline[0] = True
            while blocks_closed[0] < 2 * NQ:
                close_block()
            while sched:
                due, fn = sched.pop(0)
                fn()
            pump_evacs()
            while fill:
                fill.pop()()
            pump_evacs()
            # n1 output projection: pipeline groups so each evacuation trails
            # the next group's matmuls by one slot
            for k, (mt, dc) in enumerate([(m, d) for m in range(4) for d in range(2)]):
                po_group(1, mt, dc, tail=True)
                nc.tensor.ldweights(weights=wub[:, 0:128])
                while len(evac_q) > 1:
                    evac_q.popleft()()
            pump_evacs()

    nc.finalize()
    return nc


def prep_core_weights(g, Wq, bq, Wk, bk, Wv, bv, Wo):
    C0 = 256 * g
    wqt = np.zeros((D, 512), np.float32)
    wkt = np.zeros((D, 512), np.float32)
    wvt = np.zeros((D, 272), np.float32)
    bvrow = np.zeros((1, 272), np.float32)
    bqp = np.zeros((128, NQ), np.float32)
    bkp = np.zeros((128, NQ), np.float32)
    for t in range(NQ):
        for j in range(NJ):
            src = C0 + 64 * t + 16 * j
            wqt[:, 128 * t + 32 * j : 128 * t + 32 * j + E] = Wq[src : src + E, :].T
            wkt[:, 128 * t + 32 * j : 128 * t + 32 * j + E] = Wk[src : src + E, :].T
            m = NJ * t + j
            wvt[:, 17 * m : 17 * m + E] = Wv[src : src + E, :].T
            bvrow[0, 17 * m : 17 * m + E] = bv[src : src + E]
            bvrow[0, 17 * m + E] = 1.0
            bqp[32 * j : 32 * j + E, t] = bq[src : src + E]
            bkp[32 * j : 32 * j + E, t] = bk[src : src + E]
    wot = np.zeros((512, D), np.float32)
    for t in range(NQ):
        for j in range(NJ):
            src = C0 + 64 * t + 16 * j
            wot[128 * t + 32 * j : 128 * t + 32 * j + E, :] = Wo[:, src : src + E].T
    ind = np.zeros((4, 128), np.float32)
    sel = np.zeros((128, 4), np.float32)
    for j in range(NJ):
        ind[j, 32 * j : 32 * j + E] = 1.0
        sel[32 * j + E, j] = 1.0
    return {
        "sel": round_fp32r(sel),
        "wqt": pack_w_tmaj(fp8_np(wqt)),
        "wkt": pack_w_tmaj(fp8_np(wkt)),
        "wvt": pack_dmaj(bf16_np(wvt)),
        "wot": pack_dmaj(bf16_np(wot)),
        "bvrow": bvrow,
        "bqp": bqp,
        "bkp": bkp,
        "ind": round_fp32r(ind),
    }


def pack_dmaj(x):
    """[(a*128), c] -> [128, a*c]: partition-major with d-chunk-contiguous cols."""
    a = x.shape[0] // 128
    c = x.shape[1]
    return np.ascontiguousarray(
        x.reshape(a, 128, c).transpose(1, 0, 2).reshape(128, a * c)
    )


def pack_w_tmaj(w):
    """[(d*128), (t*128)] -> [128, t*d*128]: t-quad outer, d-chunk inner."""
    a = w.reshape(8, 128, 4, 128)
    return np.ascontiguousarray(a.transpose(1, 2, 0, 3).reshape(128, 4096))


def pack_x(x):
    """[(d*128), (h*512)] -> [128, h*d*512]: h-half outer, d-chunk inner."""
    return np.ascontiguousarray(
        x.reshape(8, 128, 2, 512).transpose(1, 2, 0, 3).reshape(128, 8192)
    )


def prep_in_maps(Q, K, V, Wq, bq, Wk, bk, Wv, bv, Wo):
    group_w = [prep_core_weights(g, Wq, bq, Wk, bk, Wv, bv, Wo) for g in range(4)]
    xt = []
    for b in range(B):
        xt.append(
            {
                "xq": pack_x(fp8_np(Q[b].T)),
                "xk": pack_x(fp8_np(K[b].T)),
                "xv": pack_x(bf16_np(V[b].T)),
            }
        )
    in_maps = []
    for c in range(8):
        b, g = c // 4, c % 4
        m = dict(group_w[g])
        m.update(xt[b])
        in_maps.append(m)
    return in_maps


def assemble_output(results, bo):
    out = np.zeros((B, S, D), np.float32)
    for b in range(B):
        acc = np.zeros((S, D), np.float64)
        for g in range(4):
            acc += results[4 * b + g]["out_part"].astype(np.float64)
        out[b] = (acc + bo.astype(np.float64)).astype(np.float32)
    return out


_NC_CACHE = {}


def _get_nc():
    if "nc" not in _NC_CACHE:
        _NC_CACHE["nc"] = build_nc()
    return _NC_CACHE["nc"]


def kernel(Q, K, V, Wq, bq, Wk, bk, Wv, bv, Wo, bo):
    import time

    from concourse.bass_utils import run_bass_kernel_spmd

    nc = _get_nc()
    in_maps = prep_in_maps(
        np.asarray(Q, np.float32),
        np.asarray(K, np.float32),
        np.asarray(V, np.float32),
        np.asarray(Wq, np.float32),
        np.asarray(bq, np.float32),
        np.asarray(Wk, np.float32),
        np.asarray(bk, np.float32),
        np.asarray(Wv, np.float32),
        np.asarray(bv, np.float32),
        np.asarray(Wo, np.float32),
    )
    # Retries: a first execution after NEFF load occasionally hits a
    # transient NRT_EXEC_UNIT_UNRECOVERABLE; re-running recovers.
    last = None
    for attempt in range(3):
        try:
            res = run_bass_kernel_spmd(nc, in_maps, list(range(8)))
            return assemble_output(res.results, np.asarray(bo, np.float32))
        except Exception as e:
            last = e
            time.sleep(3)
    raise last
